# revision 23
# baseline (speedup 1.0000x reference)
"""MixLinear int8-decomposition GEMM for Trainium2 (8 NeuronCores).

Reference semantics:
  out_mask[k]  = any(|x[:,k]| > 20)                      (outlier columns)
  w_base       = w * (1-mask);  s_c[n] = f16(max|w_base|/127) clamped
  q_w          = rint(w_base / s_c)
  s_r[m]       = max|x[m,:]|/127 clamped
  q_x          = rint(x / s_r)
  y            = (q_x @ q_w.T) * s_r * s_c + (x*mask)@(w*mask).T + bias

Implementation trick: both GEMMs are fused into ONE fp16 PE matmul.
For non-outlier columns the operands are the integer-valued q_x/q_w
(fp16 holds them exactly; fp32 PSUM accumulation is exact), for outlier
columns the operands are the UNROUNDED x/s_r and w/s_c, so that after the
(* s_r * s_c) epilogue those columns contribute x*w — the outlier GEMM.
Round-to-nearest-even is done with a per-column magic bias 1536*zmask[k]
(the fp16 +1536 trick rounds where zmask==1 and passes through where 0).

Sharding: column-parallel over N (out_features) across 8 cores; x replicated.
"""
import sys

sys.path.insert(0, '/opt/trn_rl_repo')

import numpy as np

import concourse.bass as bass
import concourse.mybir as mybir
from concourse.tile import TileContext
from concourse.vector_clock import ScopedClock, VectorClock

F16 = mybir.dt.float16
F32 = mybir.dt.float32
U16 = mybir.dt.uint16
ALU = mybir.AluOpType
ACTF = mybir.ActivationFunctionType

SIGMA = 20.0
MAGIC = 1536.0  # 1.5*2^10: fp16 round-to-nearest-even for |v| <= 127


class SplitDrainTileContext(TileContext):
    """This walrus build rejects >1 sem-wait on CTRL-type (Drain/NOP)
    instructions; pre-drain each clock lane with its own single-wait NOP and
    emit the final drain wait-free."""

    def _drain_and_barrier(self, tick_clock, wait_clock):
        gc = tick_clock.global_clock
        items = gc.items() if hasattr(gc, 'items') else [(None, gc)]
        for scope, vc in items:
            n = len(vc)
            for proc in range(n):
                t = vc[proc]
                if t > 0:
                    vec = [0] * n
                    vec[proc] = t
                    nop = self.nc.sync.nop()
                    wait_clock.add_sem_waits(
                        nop.ins, ScopedClock({scope: VectorClock(vec)}))
        self.nc.sync.drain()
        self.nc.all_engine_barrier()
        assert self.sems is not None
        popped = self.nc._tile_sem_poison_stack.pop()
        assert popped is self._sem_poison
        self.nc.clear_and_free_semaphores(list(self.sems.allocated().values()))
        self.nc.all_engine_barrier()


def _split_multi_waits(nc):
    """This walrus build rejects >1 semaphore wait per instruction (any
    struct). Hoist extra waits onto single-wait NOPs on the same engine,
    placed immediately before the instruction (engines execute their
    instructions in block order, so the stall semantics are identical)."""
    import bass_rust
    counter = [0]
    for f in nc.m.functions:
        for bb in f.blocks:
            new_list = []
            for inst in bb.instructions:
                si = inst.sync_info
                waits = list(si.on_wait) if si is not None else []
                if len(waits) > 1:
                    for wt in waits[:-1]:
                        counter[0] += 1
                        nop = mybir.InstNoOp(
                            name=f"I-waitsplit-{counter[0]}", ins=[], outs=[])
                        nop.engine = inst.engine
                        nop.sync_info = bass_rust.SyncInfo(
                            on_wait=[wt], on_update=[])
                        new_list.append(nop)
                    inst.sync_info = bass_rust.SyncInfo(
                        on_wait=[waits[-1]],
                        on_update=list(si.on_update) if si is not None else [])
                new_list.append(inst)
            bb.instructions = new_list


def build_bass(M, K, NL, n_cores=8, collective=True, split_waits=True):
    """One-core program: x[M,K]f16 (replicated), xloc[M/n_cores,K]f16 (M-shard
    for distributed stats), w[NL,K]f16 + bias[NL]f16 (N-shard) -> y[M,NL]f16.

    Pass A (activation stats: per-row max, outlier-column detection) is
    sharded over the M dim across cores; one AllGather exchanges
    rowmax chunks + partial outlier counts."""
    MT, KT, NW = M // 128, K // 128, NL // 128
    NB = K // 512  # count psum banks
    assert NL <= 512
    MLOC = M // n_cores
    MT_LOC = MLOC // 128
    CH = MLOC + K  # per-core allgather payload: rowmax chunk + cnt partial

    nc = bass.Bass(num_devices=n_cores)
    x = nc.declare_dram_parameter("x", [M, K], F16, isOutput=False)
    xloc = nc.declare_dram_parameter("xloc", [MLOC, K], F16, isOutput=False)
    w = nc.declare_dram_parameter("w", [NL, K], F16, isOutput=False)
    bias = nc.declare_dram_parameter("b", [NL], F16, isOutput=False)
    ones16 = nc.declare_dram_parameter("ones16", [1, 128], F16, isOutput=False)
    ones32 = nc.declare_dram_parameter("ones32", [1, 128], F32, isOutput=False)
    onesm = nc.declare_dram_parameter("onesm", [128, 1], F16, isOutput=False)
    y = nc.declare_dram_parameter("y", [M, NL], F16, isOutput=True)

    scr_k16 = nc.dram_tensor("scr_k16", [K], F16)
    scr_nl = nc.dram_tensor("scr_nl", [2, NL], F32)

    with SplitDrainTileContext(nc) as tc:
        with tc.tile_pool(name="const", bufs=1) as pc:
            ones16_sb = pc.tile([1, 128], F16, tag="ones16")
            ones32_sb = pc.tile([1, 128], F32, tag="ones32")
            onesm_sb = pc.tile([128, 1], F16, tag="onesm")
            bias_sb = pc.tile([1, NL], F16, tag="bias")
            nc.gpsimd.dma_start(ones16_sb[:], ones16[:])
            nc.gpsimd.dma_start(ones32_sb[:], ones32[:])
            nc.gpsimd.dma_start(onesm_sb[:], onesm[:])
            nc.gpsimd.dma_start(bias_sb[0:1, :], bias[:])

            neg20 = pc.tile([128, 1], F32, tag="neg20")
            nc.vector.memset(neg20[:], -SIGMA)
            corr_hi = pc.tile([1, NL], F16, tag="corr_hi")
            corr_lo = pc.tile([1, NL], F16, tag="corr_lo")
            corr2 = pc.tile([2, NL], F16, tag="corr2")
            neg_magic2 = pc.tile([2, 128], F16, tag="neg_magic2")
            nc.vector.memset(neg_magic2[:], -MAGIC)
            s_r = pc.tile([128, MT], F32, tag="s_r")
            s_recip = pc.tile([128, MT], F32, tag="s_recip")
            magicz128 = pc.tile([128, K], F16, tag="magicz128")
            scol128 = pc.tile([128, NL], F32, tag="scol128")
            bias128 = pc.tile([128, NL], F32, tag="bias128")
            qwT = pc.tile([128, KT, NL], F16, tag="qwT")

            with tc.tile_pool(name="wpool", bufs=1) as pw:
                rowmax = pw.tile([128, MT], F16, tag="rowmax")
                zmask128 = pw.tile([128, K], F16, tag="zmask128")
                screcipT = pw.tile([128, NL], F32, tag="screcipT")
                zmaskT = pw.tile([128, KT], F16, tag="zmaskT")
                magiczT = pw.tile([128, KT], F32, tag="magiczT")
                zmask_row = pw.tile([1, K], F16, tag="zmask_row")
                wT = pw.tile([128, KT, NL], F16, tag="wT")

                # ------- pass A: activation stats on the local M-shard -------
                rowmax_loc = pw.tile([128, MT_LOC], F16, tag="rowmax_loc")
                cnt_row_loc = pw.tile([1, K], F16, tag="cnt_row_loc")
                cg = pw.tile([32, K], F16, tag="cg")
                ones8 = pw.tile([32, 1], F16, tag="ones8")
                nc.vector.memset(ones8[:], 1.0)
                with tc.tile_pool(name="psA", bufs=1,
                                  space=bass.MemorySpace.PSUM) as psA:
                    cnt_ps = [psA.tile([1, 512], F32, tag=f"cnt{b}",
                                       name=f"cnt{b}") for b in range(NB)]
                    with tc.tile_pool(name="pA", bufs=3) as pa:
                        for mt in range(MT_LOC):
                            x_t = pa.tile([128, K], F16, tag="x_t")
                            nc.gpsimd.dma_start(
                                x_t[:], xloc[mt * 128:(mt + 1) * 128, :])
                            abs_t = pa.tile([128, K], F16, tag="abs_t", bufs=2)
                            nc.vector.tensor_scalar(
                                out=abs_t.bitcast(U16)[:],
                                in0=x_t.bitcast(U16)[:],
                                scalar1=0x7FFF, scalar2=None,
                                op0=ALU.bitwise_and)
                            # rowmax via 3 pairwise max folds + small reduce
                            fold = pa.tile([128, K // 2], F16, tag="fold", bufs=1)
                            nc.vector.tensor_max(
                                fold[:, 0:K // 2],
                                abs_t[:, 0:K // 2], abs_t[:, K // 2:K])
                            nc.vector.tensor_max(
                                fold[:, 0:K // 4],
                                fold[:, 0:K // 4], fold[:, K // 4:K // 2])
                            nc.vector.tensor_max(
                                fold[:, 0:K // 8],
                                fold[:, 0:K // 8], fold[:, K // 8:K // 4])
                            nc.vector.tensor_reduce(
                                out=rowmax_loc[:, mt:mt + 1],
                                in_=fold[:, 0:K // 8],
                                axis=mybir.AxisListType.X, op=ALU.max)
                            # relu(|x|-20): column-sum > 0 iff outlier column
                            ind_t = pa.tile([128, K], F16, tag="ind_t", bufs=2)
                            nc.scalar.activation(ind_t[:], abs_t[:], ACTF.Relu,
                                                 bias=neg20[:], scale=1.0)
                            for b in range(NB):
                                nc.tensor.matmul(
                                    cnt_ps[b][:],
                                    onesm_sb[:],
                                    ind_t[:, b * 512:(b + 1) * 512],
                                    start=(mt == 0), stop=(mt == MT_LOC - 1))
                    for b in range(NB):
                        nc.vector.tensor_copy(
                            cnt_row_loc[:, b * 512:(b + 1) * 512], cnt_ps[b][:])

                # w.T via one batched xbar transpose: wT[q, t, n] = w[n, t*128+q]
                # (issued after pass-A loads so its completion sem doesn't
                # stall the gpsimd DMA queue at startup)
                nc.sync.dma_start_transpose(out=wT[:, :, :], in_=w[0:NL, 0:K])

                # -------- exchange stats: AllGather rowmax + cnt partials ----
                with tc.tile_pool(name="dramA", bufs=1, space="DRAM") as dra:
                    scr_in = dra.tile([CH], F16)
                    scr_out = dra.tile([n_cores * CH], F16)
                    nc.gpsimd.dma_start(
                        scr_in[0:MLOC].rearrange("(p t) -> p t", t=MT_LOC),
                        rowmax_loc[:])
                    nc.gpsimd.dma_start(scr_in[MLOC:CH], cnt_row_loc[0:1, :])
                    if collective:
                        nc.gpsimd.collective_compute(
                            "AllGather", ALU.bypass,
                            replica_groups=[list(range(n_cores))],
                            ins=[scr_in.opt()], outs=[scr_out.opt()])
                    else:
                        # sim-only stand-in (TimelineSim can't model CCs)
                        for j in range(n_cores):
                            nc.gpsimd.dma_start(
                                scr_out[j * CH:(j + 1) * CH], scr_in[:])
                    v = scr_out[:].rearrange("(j f) -> j f", j=n_cores)
                    nc.gpsimd.dma_start(cg[0:n_cores, :], v[:, MLOC:CH])
                    nc.gpsimd.dma_start(
                        rowmax[:].rearrange("p (j t) -> p j t", j=n_cores),
                        v[:, 0:MLOC].rearrange("j (p t) -> p j t", t=MT_LOC))
                # global cnt = column-sum of the 8 partial rows (fp32 matmul)
                with tc.tile_pool(name="psA2", bufs=1,
                                  space=bass.MemorySpace.PSUM) as psA2:
                    for b in range(NB):
                        cps = psA2.tile([1, 512], F32, tag="cps")
                        nc.tensor.matmul(
                            cps[:], ones8[0:n_cores, :],
                            cg[0:n_cores, b * 512:(b + 1) * 512],
                            start=True, stop=True)
                        nc.vector.tensor_scalar(
                            out=zmask_row[:, b * 512:(b + 1) * 512],
                            in0=cps[:], scalar1=1e-3, scalar2=None,
                            op0=ALU.is_lt)

                # ------------- finale: masks, scales, q_w' -------------
                # zmaskT[p, t] = zmask[t*128+p] via DRAM round-trip
                nc.gpsimd.dma_start(scr_k16[:], zmask_row[0:1, :])
                nc.gpsimd.dma_start(
                    zmaskT[:], scr_k16[:].rearrange("(t p) -> p t", p=128))
                nc.vector.tensor_scalar(out=magiczT[:], in0=zmaskT[:],
                                        scalar1=MAGIC, scalar2=None,
                                        op0=ALU.mult)

                # s_r from rowmax
                inv127 = float(np.float32(1.0) / np.float32(127.0))
                nc.vector.tensor_scalar(out=s_r[:], in0=rowmax[:],
                                        scalar1=inv127, scalar2=1e-8,
                                        op0=ALU.mult, op1=ALU.max)
                nc.vector.reciprocal(s_recip[:], s_r[:])

                with tc.tile_pool(name="psF", bufs=2,
                                  space=bass.MemorySpace.PSUM) as psF:
                    with tc.tile_pool(name="pF", bufs=2) as pf:
                        # zmask128 = ones (x) zmask_row; magicz128 = 1536*z
                        for b in range(NB):
                            bc = psF.tile([128, 512], F32, tag="bc")
                            nc.tensor.matmul(
                                bc[:], ones16_sb[:],
                                zmask_row[:, b * 512:(b + 1) * 512],
                                start=True, stop=True)
                            nc.vector.tensor_copy(
                                zmask128[:, b * 512:(b + 1) * 512], bc[:])
                        nc.vector.tensor_scalar(
                            out=magicz128[:], in0=zmask128[:],
                            scalar1=MAGIC, scalar2=None, op0=ALU.mult)

                        # bias128 = ones (x) bias
                        bcb = psF.tile([128, NL], F32, tag="bcb")
                        nc.tensor.matmul(bcb[:], ones16_sb[:], bias_sb[:],
                                         start=True, stop=True)
                        nc.vector.tensor_copy(bias128[:], bcb[:])

                        # scale_col from natural-layout w (streamed)
                        wmax = pf.tile([128, NW], F32, tag="wmax")
                        for i in range(NW):
                            wnat_t = pf.tile([128, K], F16, tag="wnat_t")
                            nc.gpsimd.dma_start(
                                wnat_t[:], w[i * 128:(i + 1) * 128, :])
                            wb = pf.tile([128, K], F16, tag="wb")
                            nc.vector.tensor_mul(wb[:], wnat_t[:],
                                                 zmask128[:])
                            nc.vector.tensor_reduce(
                                out=wmax[:, i:i + 1], in_=wb[:],
                                axis=mybir.AxisListType.X, op=ALU.max,
                                apply_absolute_value=True)
                        s_c = pf.tile([128, NW], F32, tag="s_c")
                        s_c16 = pf.tile([128, NW], F16, tag="s_c16")
                        nc.vector.tensor_scalar(out=s_c[:], in0=wmax[:],
                                                scalar1=float(np.float32(1.0) / np.float32(127.0)),
                                                scalar2=None, op0=ALU.mult)
                        nc.vector.tensor_copy(s_c16[:], s_c[:])
                        nc.vector.tensor_copy(s_c[:], s_c16[:])
                        nc.vector.tensor_scalar(out=s_c[:], in0=s_c[:],
                                                scalar1=1e-8, scalar2=None,
                                                op0=ALU.max)
                        s_cr = pf.tile([128, NW], F32, tag="s_cr")
                        nc.vector.reciprocal(s_cr[:], s_c[:])
                        # flatten both to [1, NL] rows via DRAM
                        nc.gpsimd.dma_start(
                            scr_nl[0].rearrange("(t p) -> p t", p=128), s_c[:])
                        nc.gpsimd.dma_start(
                            scr_nl[1].rearrange("(t p) -> p t", p=128), s_cr[:])
                        scol_row = pf.tile([1, NL], F32, tag="scol_row")
                        scr_row = pf.tile([1, NL], F32, tag="scr_row")
                        nc.gpsimd.dma_start(scol_row[0:1, :], scr_nl[0])
                        nc.gpsimd.dma_start(scr_row[0:1, :], scr_nl[1])
                        bc2 = psF.tile([128, NL], F32, tag="bc2")
                        nc.tensor.matmul(bc2[:], ones32_sb[:], scol_row[:],
                                         start=True, stop=True)
                        nc.vector.tensor_copy(scol128[:], bc2[:])
                        bc3 = psF.tile([128, NL], F32, tag="bc2")
                        nc.tensor.matmul(bc3[:], ones32_sb[:], scr_row[:],
                                         start=True, stop=True)
                        nc.vector.tensor_copy(screcipT[:], bc3[:])

                        # q_w' in transposed layout:
                        #   rint(w/s_c) on zmask cols, raw w/s_c on outliers
                        for kt in range(KT):
                            qst = pf.tile([128, NL], F32, tag="qst")
                            nc.vector.tensor_mul(qst[:], wT[:, kt, :],
                                                 screcipT[:])
                            qr = pf.tile([128, NL], F16, tag="qr")
                            nc.scalar.activation(
                                qr[:], qst[:], ACTF.Identity,
                                bias=magiczT[:, kt:kt + 1], scale=1.0)
                            nc.vector.tensor_scalar(
                                out=qwT[:, kt, :], in0=qr[:],
                                scalar1=magiczT[:, kt:kt + 1], scalar2=None,
                                op0=ALU.subtract)

                        # S[n] = sum_{k: zmask=1} qw[k,n]  (exact ints in f32)
                        # split S = Shi + Slo with both f16-exact so a rank-2
                        # fp16 matmul can subtract 1536*S from PSUM exactly.
                        psS = psF.tile([1, NL], F32, tag="psS")
                        for kt in range(KT):
                            nc.tensor.matmul(
                                psS[:], zmaskT[:, kt:kt + 1], qwT[:, kt, :],
                                start=(kt == 0), stop=(kt == KT - 1))
                        nc.vector.tensor_copy(corr_hi[:], psS[:])
                        nc.vector.tensor_sub(corr_lo[:], psS[:], corr_hi[:])
                        # assemble [2, NL] rhs via DMA (engines can't write
                        # partition offset 1 directly)
                        nc.gpsimd.dma_start(corr2[0:1, :], corr_hi[0:1, :])
                        nc.gpsimd.dma_start(corr2[1:2, :], corr_lo[0:1, :])

            # ---------------- pass B: quantize + fused GEMM ----------------
            # software-pipelined: quantize+transpose run one m-tile ahead of
            # the GEMM+epilogue so queue order never blocks the PE.
            with tc.tile_pool(name="psB", bufs=3,
                              space=bass.MemorySpace.PSUM) as psB:
                with tc.tile_pool(name="pB", bufs=3) as pb:
                    with tc.tile_pool(name="pB2", bufs=2) as pb2:
                        def quant_stage(mt):
                            x_t = pb.tile([128, K], F16, tag="x_t")
                            nc.gpsimd.dma_start(
                                x_t[:], x[mt * 128:(mt + 1) * 128, :])
                            # q_b = x/s_r + 1536*zmask  (fp16 out: rint where
                            # zmask==1, raw value on outlier columns)
                            qb = pb2.tile([128, K], F16, tag="qb", bufs=3)
                            nc.vector.scalar_tensor_tensor(
                                out=qb[:], in0=x_t[:],
                                scalar=s_recip[:, mt:mt + 1],
                                in1=magicz128[:],
                                op0=ALU.mult, op1=ALU.add)
                            # batched xbar: qT[q, t, p] = qb[p, t*128+q]
                            # (magic bias left in; removed by corr matmul)
                            qT = pb2.tile([128, KT, 128], F16, tag="qT",
                                          bufs=3)
                            nc.sync.dma_start_transpose(out=qT[:, :, :],
                                                        in_=qb[:, 0:K])
                            return qT

                        def gemm_stage(mt, qT):
                            py = psB.tile([128, NL], F32, tag="py")
                            for kt in range(KT):
                                nc.tensor.matmul(
                                    py[:], qT[:, kt, :], qwT[:, kt, :],
                                    start=(kt == 0), stop=False)
                            # py -= 1536*(Shi+Slo) = 1536*sum_z qw per column
                            nc.tensor.matmul(py[:], neg_magic2[:], corr2[:],
                                             start=False, stop=True)
                            t2e = pb2.tile([128, NL], F32, tag="t2e")
                            nc.vector.scalar_tensor_tensor(
                                out=t2e[:], in0=py[:],
                                scalar=s_r[:, mt:mt + 1], in1=scol128[:],
                                op0=ALU.mult, op1=ALU.mult)
                            y_t = pb2.tile([128, NL], F16, tag="y_t")
                            nc.vector.tensor_add(y_t[:], t2e[:], bias128[:])
                            nc.scalar.dma_start(
                                y[mt * 128:(mt + 1) * 128, :], y_t[:])

                        qT_prev = quant_stage(0)
                        for mt in range(1, MT):
                            qT_cur = quant_stage(mt)
                            gemm_stage(mt - 1, qT_prev)
                            qT_prev = qT_cur
                        gemm_stage(MT - 1, qT_prev)
    if split_waits:
        _split_multi_waits(nc)
    return nc


def make_consts():
    return {
        "ones16": np.ones((1, 128), dtype=np.float16),
        "ones32": np.ones((1, 128), dtype=np.float32),
        "onesm": np.ones((128, 1), dtype=np.float16),
    }


_CACHE = {}


def kernel(x, weight, bias):
    from concourse.bass_utils import run_bass_kernel_spmd

    B, S, K = x.shape
    N = weight.shape[0]
    M = B * S
    NC = 8
    NL = N // NC

    key = (M, K, NL)
    if key not in _CACHE:
        _CACHE[key] = build_bass(M, K, NL, n_cores=NC)
    nc = _CACHE[key]

    consts = make_consts()
    xf = np.ascontiguousarray(x.reshape(M, K))
    MLOC = M // NC
    in_maps = []
    for c in range(NC):
        m = dict(consts)
        m["x"] = xf
        m["xloc"] = np.ascontiguousarray(xf[c * MLOC:(c + 1) * MLOC, :])
        m["w"] = np.ascontiguousarray(weight[c * NL:(c + 1) * NL, :])
        m["b"] = np.ascontiguousarray(bias[c * NL:(c + 1) * NL])
        in_maps.append(m)

    res = run_bass_kernel_spmd(nc, in_maps, core_ids=list(range(NC)))
    y = np.concatenate([res.results[c]["y"] for c in range(NC)], axis=1)
    return y.reshape(B, S, N).astype(np.float16)

